# revision 1
# baseline (speedup 1.0000x reference)
"""Expert-parallel MoE MLP (8 experts -> 8 NeuronCores) Bass kernel for TRN2.

Problem: y[t] = W2[e] @ gelu(W1[e] @ x[t] + b1[e]) + b2[e], tokens contiguous
per expert, 2048 tokens/expert, d_in=d_out=1024, d_hid=4096.

Sharding: expert-parallel. Core e gets expert e's weights and its 2048 tokens.
No cross-core communication needed (counts are equal and tokens are already
sorted by expert); host does the shard/unshard.

Per-core compute layout (everything partition-major, h kept as [hid, tok]):
  GEMM1: h[hid, tok]  = w1T[k,:].T @ xT[k, tok]   (accum over k = d_in tiles)
  GELU : h = gelu(h + b1)  via ScalarE with fused per-partition bias
  GEMM2: y[dout, tok] = w2T[k,:].T @ h[k, tok]    (accum over k = d_hid tiles)
  BIAS : y += b2 via ScalarE Identity with fused bias

Weights/acts are fp16 on device (fp32 PSUM accumulation); host pre-permutes
w1/w2/x into the exact SBUF layouts so every DMA moves large contiguous
packets, split across queues so all 16 DMA engines pull in parallel, ordered
so the first GEMM1 tiles' data lands first.
"""
import sys

sys.path.insert(0, "/opt/trn_rl_repo")

import numpy as np

import concourse.bass as bass  # noqa: F401
import concourse.tile as tile
from concourse import bacc, mybir
from concourse.bass_utils import run_bass_kernel_spmd

E = 8
T_PER_E = 2048
D_IN = 1024
D_HID = 4096
D_OUT = 1024

TOK_BLK = 512          # tokens per block (= PSUM bank free size in fp32)
N_TOK_BLK = T_PER_E // TOK_BLK
K1 = D_IN // 128       # k tiles for GEMM1
M1 = D_HID // 128      # output row tiles for GEMM1
K2 = D_HID // 128      # k tiles for GEMM2
M2 = D_OUT // 128      # output row tiles for GEMM2

CDT = mybir.dt.float16   # compute dtype on device (weights + activations)
NP_CDT = np.float16

N_WARM = 64              # dummy matmuls to flip the PE HAM clock gate early

_compiled = None


def _build():
    nc = bacc.Bacc("TRN2", target_bir_lowering=False, debug=False)

    # Host-permuted layouts (see _make_in_maps):
    #   xL [128, t*4096 + k*512 + c]   = x[t*512+c, k*128+p]
    #   w1L[128, m*1024 + k*128 + mc]  = w1[m*128+mc, k*128+p]
    #   w2L[128, d*4096 + k*128 + dc]  = w2[d*128+dc, k*128+p]
    xL = nc.dram_tensor("xL", [128, N_TOK_BLK * K1 * TOK_BLK], CDT, kind="ExternalInput").ap()
    w1L = nc.dram_tensor("w1L", [128, M1 * K1 * 128], CDT, kind="ExternalInput").ap()
    w2L = nc.dram_tensor("w2L", [128, M2 * K2 * 128], CDT, kind="ExternalInput").ap()
    b1r = nc.dram_tensor("b1r", [128, M1], mybir.dt.float32, kind="ExternalInput").ap()
    b2r = nc.dram_tensor("b2r", [128, M2], mybir.dt.float32, kind="ExternalInput").ap()
    yT = nc.dram_tensor("yT", [D_OUT, T_PER_E], mybir.dt.float32, kind="ExternalOutput").ap()

    XBLK = K1 * TOK_BLK  # 4096 cols per token block in xL

    with tile.TileContext(nc) as tc:
        with tc.tile_pool(name="wpool", bufs=1) as wpool, \
             tc.tile_pool(name="xpool", bufs=2) as xpool, \
             tc.tile_pool(name="hpool", bufs=1) as hpool, \
             tc.tile_pool(name="opool", bufs=4) as opool, \
             tc.tile_pool(name="ps1", bufs=3, space="PSUM") as ps1, \
             tc.tile_pool(name="ps2", bufs=4, space="PSUM") as ps2:

            # --- PE warmup: flip the HAM clock gate while DMAs stream in ---
            scr = wpool.tile([128, 128], CDT, name="scr")
            nc.vector.memset(scr[:], 0.0)
            for i in range(N_WARM):
                wps = ps1.tile([128, 128], mybir.dt.float32, tag="ps1", name=f"warm{i}")
                nc.tensor.matmul(wps[:], scr[:], scr[:], start=True, stop=True)

            # --- biases first: tiny, and the gelu PSUM evacuation (and with
            # it the whole PE pipeline) blocks until b1 is resident ---
            b1_sb = wpool.tile([128, M1], mybir.dt.float32, name="b1_sb")
            nc.sync.dma_start(b1_sb[:], b1r[:, :])
            b2_sb = wpool.tile([128, M2], mybir.dt.float32, name="b2_sb")
            nc.sync.dma_start(b2_sb[:], b2r[:, :])

            # --- prologue DMAs: x(t0) split 8 ways, then w1 per m-tile ---
            x_blocks = {}
            x_sb = xpool.tile([128, XBLK], CDT, tag="x", name="x_sb0")
            for j in range(8):
                w = XBLK // 8
                nc.sync.dma_start(x_sb[:, j * w:(j + 1) * w], xL[:, j * w:(j + 1) * w])
            x_blocks[0] = x_sb

            # w1 per m-tile, split in 2 (128 KB pieces) so each m-tile lands
            # with ~5 us latency across two DMA engines, in consumption order
            w1_sb = wpool.tile([128, M1 * K1 * 128], CDT, name="w1_sb")
            mw = K1 * 128
            for m in range(M1):
                for j in range(2):
                    lo = m * mw + j * (mw // 2)
                    hi = m * mw + (j + 1) * (mw // 2)
                    nc.sync.dma_start(w1_sb[:, lo:hi], w1L[:, lo:hi])

            # x(t=1) before w2: both are needed when block 1 / GEMM2(t0) start,
            # but x1 is needed slightly earlier and is much smaller
            x_sb1 = xpool.tile([128, XBLK], CDT, tag="x", name="x_sb1")
            for j in range(4):
                w = XBLK // 4
                nc.sync.dma_start(x_sb1[:, j * w:(j + 1) * w],
                                  xL[:, XBLK + j * w: XBLK + (j + 1) * w])
            x_blocks[1] = x_sb1

            # w2 in one piece per d-tile: it lands ~60us before GEMM2 needs it,
            # and fewer DMA pieces keep the PE wait/drain overhead down
            w2_sb = wpool.tile([128, M2 * K2 * 128], CDT, name="w2_sb")
            dw = K2 * 128
            for d in range(M2):
                nc.sync.dma_start(w2_sb[:, d * dw:(d + 1) * dw],
                                  w2L[:, d * dw:(d + 1) * dw])

            for t in range(N_TOK_BLK):
                if t in x_blocks:
                    x_sb = x_blocks[t]
                else:
                    x_sb = xpool.tile([128, XBLK], CDT, tag="x", name=f"x_sb{t}")
                    for j in range(4):
                        w = XBLK // 4
                        nc.sync.dma_start(x_sb[:, j * w:(j + 1) * w],
                                          xL[:, t * XBLK + j * w: t * XBLK + (j + 1) * w])

                # --- GEMM1 + gelu: h[m] tiles ---
                h_tiles = []
                for m in range(M1):
                    psum = ps1.tile([128, TOK_BLK], mybir.dt.float32,
                                    tag="ps1", name=f"ps1_{t}_{m}")
                    for k in range(K1):
                        nc.tensor.matmul(
                            psum[:],
                            w1_sb[:, m * (K1 * 128) + k * 128: m * (K1 * 128) + (k + 1) * 128],
                            x_sb[:, k * TOK_BLK:(k + 1) * TOK_BLK],
                            start=(k == 0), stop=(k == K1 - 1),
                        )
                    h_sb = hpool.tile([128, TOK_BLK], CDT, tag=f"h{m}",
                                      name=f"h_sb{t}_{m}")
                    nc.scalar.activation(h_sb[:], psum[:],
                                         mybir.ActivationFunctionType.Gelu,
                                         bias=b1_sb[:, m:m + 1], scale=1.0)
                    h_tiles.append(h_sb)

                # --- GEMM2 + bias: y[d] tiles ---
                for d in range(M2):
                    psum = ps2.tile([128, TOK_BLK], mybir.dt.float32,
                                    tag="ps2", name=f"ps2_{t}_{d}")
                    for k in range(K2):
                        nc.tensor.matmul(
                            psum[:],
                            w2_sb[:, d * (K2 * 128) + k * 128: d * (K2 * 128) + (k + 1) * 128],
                            h_tiles[k][:],
                            start=(k == 0), stop=(k == K2 - 1),
                        )
                    o_sb = opool.tile([128, TOK_BLK], mybir.dt.float32,
                                      tag="o", name=f"o_sb{t}_{d}")
                    nc.scalar.activation(o_sb[:], psum[:],
                                         mybir.ActivationFunctionType.Identity,
                                         bias=b2_sb[:, d:d + 1], scale=1.0)
                    nc.sync.dma_start(yT[d * 128:(d + 1) * 128,
                                         t * TOK_BLK:(t + 1) * TOK_BLK],
                                      o_sb[:])

    nc.compile()
    return nc


def _get_compiled():
    global _compiled
    if _compiled is None:
        _compiled = _build()
    return _compiled


def _make_in_maps(x, w1, b1, w2, b2):
    in_maps = []
    for e in range(E):
        xe = x[e * T_PER_E:(e + 1) * T_PER_E]            # [2048, 1024]
        xl = xe.reshape(N_TOK_BLK, TOK_BLK, K1, 128)     # t, c, k, p
        xl = xl.transpose(3, 0, 2, 1).reshape(128, -1)   # p, (t k c)
        w1e = w1[e].reshape(M1, 128, K1, 128)            # m, mc, k, p
        w1l = w1e.transpose(3, 0, 2, 1).reshape(128, -1)  # p, (m k mc)
        w2e = w2[e].reshape(M2, 128, K2, 128)            # d, dc, k, p
        w2l = w2e.transpose(3, 0, 2, 1).reshape(128, -1)  # p, (d k dc)
        in_maps.append({
            "xL": np.ascontiguousarray(xl).astype(NP_CDT),
            "w1L": np.ascontiguousarray(w1l).astype(NP_CDT),
            "w2L": np.ascontiguousarray(w2l).astype(NP_CDT),
            "b1r": np.ascontiguousarray(b1[e].reshape(M1, 128).T).astype(np.float32),
            "b2r": np.ascontiguousarray(b2[e].reshape(M2, 128).T).astype(np.float32),
        })
    return in_maps


def run(x, cnt, w1, b1, w2, b2, trace=False):
    nc = _get_compiled()
    in_maps = _make_in_maps(x, w1, b1, w2, b2)
    res = run_bass_kernel_spmd(nc, in_maps, core_ids=list(range(E)), trace=trace)
    outs = [res.results[e]["yT"].T for e in range(E)]
    y = np.concatenate(outs, axis=0).astype(np.float32)
    return y, res


def kernel(x, cnt, w1, b1, w2, b2):
    y, _ = run(x, cnt, w1, b1, w2, b2, trace=False)
    return y



# revision 2
# speedup vs baseline: 1.0000x; 1.0000x over previous
"""Expert-parallel MoE MLP (8 experts -> 8 NeuronCores) Bass kernel for TRN2.

Problem: y[t] = W2[e] @ gelu(W1[e] @ x[t] + b1[e]) + b2[e], tokens contiguous
per expert, 2048 tokens/expert, d_in=d_out=1024, d_hid=4096.

Sharding: expert-parallel. Core e gets expert e's weights and its 2048 tokens.
No cross-core communication needed (counts are equal and tokens are already
sorted by expert); host does the shard/unshard.

Per-core compute layout (everything partition-major, h kept as [hid, tok]):
  GEMM1: h[hid, tok]  = w1T[k,:].T @ xT[k, tok]   (accum over k = d_in tiles)
  GELU : h = gelu(h + b1)  via ScalarE with fused per-partition bias
  GEMM2: y[dout, tok] = w2T[k,:].T @ h[k, tok]    (accum over k = d_hid tiles)
  BIAS : y += b2 via ScalarE Identity with fused bias

Timing model (measured): ~7us framework preamble gates everything, first DMA
byte ~8.6us, steady-state MM spacing 215.6ns (= silicon floor for N=512 bf16).
So the only addressable time is the ramp-in (first-tile data latency), keeping
HAM warm across the ramp, and the serial tail after the last matmul.

v2 changes vs v1:
  - weights stream on the sync HWDGE ring, x blocks + y writebacks on the
    scalar HWDGE ring: the two physical rings drain in parallel, and the
    prologue-critical bytes (w1 m0 + x0 k0-1) are first on each ring.
  - fewer, larger DMA pieces (each dma_start costs ~0.6us dispatch on the
    issuing engine; v1's 8-way x0 split delayed w1's first byte by ~5us).
  - warmup matmuls sized to bridge PE activity until real data arrives
    (HAM stays at K=8/8; v1 had a 2.5us gap at ~14us that re-throttled
    the PE clock to 1.2GHz for 3.4us).
  - last output tile computed as two 256-column PSUM groups so the final
    activation+DMA overlaps the last 32 matmuls instead of serializing.
"""
import sys

sys.path.insert(0, "/opt/trn_rl_repo")

import numpy as np

import concourse.bass as bass  # noqa: F401
import concourse.tile as tile
from concourse import bacc, mybir
from concourse.bass_utils import run_bass_kernel_spmd

E = 8
T_PER_E = 2048
D_IN = 1024
D_HID = 4096
D_OUT = 1024

TOK_BLK = 512          # tokens per block (= PSUM bank free size in fp32)
N_TOK_BLK = T_PER_E // TOK_BLK
K1 = D_IN // 128       # k tiles for GEMM1
M1 = D_HID // 128      # output row tiles for GEMM1
K2 = D_HID // 128      # k tiles for GEMM2
M2 = D_OUT // 128      # output row tiles for GEMM2

CDT = mybir.dt.float16   # compute dtype on device (weights + activations)
NP_CDT = np.float16

N_WARM = 40              # N=512 warmup matmuls bridging until first data lands

_compiled = None


def _build():
    nc = bacc.Bacc("TRN2", target_bir_lowering=False, debug=False)

    # Host-permuted layouts (see _make_in_maps):
    #   xL [128, t*4096 + k*512 + c]   = x[t*512+c, k*128+p]
    #   w1L[128, m*1024 + k*128 + mc]  = w1[m*128+mc, k*128+p]
    #   w2L[128, d*4096 + k*128 + dc]  = w2[d*128+dc, k*128+p]
    xL = nc.dram_tensor("xL", [128, N_TOK_BLK * K1 * TOK_BLK], CDT, kind="ExternalInput").ap()
    w1L = nc.dram_tensor("w1L", [128, M1 * K1 * 128], CDT, kind="ExternalInput").ap()
    w2L = nc.dram_tensor("w2L", [128, M2 * K2 * 128], CDT, kind="ExternalInput").ap()
    b1r = nc.dram_tensor("b1r", [128, M1], mybir.dt.float32, kind="ExternalInput").ap()
    b2r = nc.dram_tensor("b2r", [128, M2], mybir.dt.float32, kind="ExternalInput").ap()
    yT = nc.dram_tensor("yT", [D_OUT, T_PER_E], mybir.dt.float32, kind="ExternalOutput").ap()

    XBLK = K1 * TOK_BLK  # 4096 cols per token block in xL
    mw = K1 * 128        # 1024 cols per w1 m-tile
    dw = K2 * 128        # 4096 cols per w2 d-tile

    with tile.TileContext(nc) as tc:
        with tc.tile_pool(name="wpool", bufs=1) as wpool, \
             tc.tile_pool(name="xpool", bufs=2) as xpool, \
             tc.tile_pool(name="hpool", bufs=1) as hpool, \
             tc.tile_pool(name="opool", bufs=4) as opool, \
             tc.tile_pool(name="ps1", bufs=3, space="PSUM") as ps1, \
             tc.tile_pool(name="ps2", bufs=4, space="PSUM") as ps2:

            # --- PE warmup: keep the HAM clock gate warm until data lands ---
            scr = wpool.tile([128, TOK_BLK], CDT, name="scr")
            nc.vector.memset(scr[:], 0.0)
            for i in range(N_WARM):
                wps = ps1.tile([128, TOK_BLK], mybir.dt.float32, tag="ps1", name=f"warm{i}")
                nc.tensor.matmul(wps[:], scr[:, :128], scr[:], start=True, stop=True)

            # --- sync ring: biases (tiny), then w1 m-tile by m-tile ---
            b1_sb = wpool.tile([128, M1], mybir.dt.float32, name="b1_sb")
            nc.sync.dma_start(b1_sb[:], b1r[:, :])
            b2_sb = wpool.tile([128, M2], mybir.dt.float32, name="b2_sb")
            nc.sync.dma_start(b2_sb[:], b2r[:, :])

            w1_sb = wpool.tile([128, M1 * mw], CDT, name="w1_sb")
            for m in range(M1):
                nc.sync.dma_start(w1_sb[:, m * mw:(m + 1) * mw],
                                  w1L[:, m * mw:(m + 1) * mw])

            # --- scalar ring (parallel with sync ring): x blocks ---
            x_blocks = {}
            x_sb = xpool.tile([128, XBLK], CDT, tag="x", name="x_sb0")
            for j in range(4):           # 2 k-slices per piece: m-groups ramp in
                w = XBLK // 4
                nc.scalar.dma_start(x_sb[:, j * w:(j + 1) * w], xL[:, j * w:(j + 1) * w])
            x_blocks[0] = x_sb

            x_sb1 = xpool.tile([128, XBLK], CDT, tag="x", name="x_sb1")
            for j in range(2):
                w = XBLK // 2
                nc.scalar.dma_start(x_sb1[:, j * w:(j + 1) * w],
                                    xL[:, XBLK + j * w: XBLK + (j + 1) * w])
            x_blocks[1] = x_sb1

            # --- sync ring: w2, one d-tile per piece ---
            w2_sb = wpool.tile([128, M2 * dw], CDT, name="w2_sb")
            for d in range(M2):
                nc.sync.dma_start(w2_sb[:, d * dw:(d + 1) * dw],
                                  w2L[:, d * dw:(d + 1) * dw])

            for t in range(N_TOK_BLK):
                if t in x_blocks:
                    x_sb = x_blocks[t]
                else:
                    x_sb = xpool.tile([128, XBLK], CDT, tag="x", name=f"x_sb{t}")
                    for j in range(2):
                        w = XBLK // 2
                        nc.scalar.dma_start(x_sb[:, j * w:(j + 1) * w],
                                            xL[:, t * XBLK + j * w: t * XBLK + (j + 1) * w])

                # --- GEMM1 + gelu: h[m] tiles ---
                h_tiles = []
                for m in range(M1):
                    psum = ps1.tile([128, TOK_BLK], mybir.dt.float32,
                                    tag="ps1", name=f"ps1_{t}_{m}")
                    for k in range(K1):
                        nc.tensor.matmul(
                            psum[:],
                            w1_sb[:, m * mw + k * 128: m * mw + (k + 1) * 128],
                            x_sb[:, k * TOK_BLK:(k + 1) * TOK_BLK],
                            start=(k == 0), stop=(k == K1 - 1),
                        )
                    h_sb = hpool.tile([128, TOK_BLK], CDT, tag=f"h{m}",
                                      name=f"h_sb{t}_{m}")
                    nc.scalar.activation(h_sb[:], psum[:],
                                         mybir.ActivationFunctionType.Gelu,
                                         bias=b1_sb[:, m:m + 1], scale=1.0)
                    h_tiles.append(h_sb)

                # --- GEMM2 + bias: y[d] tiles ---
                for d in range(M2):
                    last = (t == N_TOK_BLK - 1 and d == M2 - 1)
                    if not last:
                        psum = ps2.tile([128, TOK_BLK], mybir.dt.float32,
                                        tag="ps2", name=f"ps2_{t}_{d}")
                        for k in range(K2):
                            nc.tensor.matmul(
                                psum[:],
                                w2_sb[:, d * dw + k * 128: d * dw + (k + 1) * 128],
                                h_tiles[k][:],
                                start=(k == 0), stop=(k == K2 - 1),
                            )
                        o_sb = opool.tile([128, TOK_BLK], mybir.dt.float32,
                                          tag="o", name=f"o_sb{t}_{d}")
                        nc.scalar.activation(o_sb[:], psum[:],
                                             mybir.ActivationFunctionType.Identity,
                                             bias=b2_sb[:, d:d + 1], scale=1.0)
                        nc.scalar.dma_start(yT[d * 128:(d + 1) * 128,
                                               t * TOK_BLK:(t + 1) * TOK_BLK],
                                            o_sb[:])
                    else:
                        # final tile: two half-column groups so the last
                        # activation+DMA overlaps the second group's matmuls
                        for half in range(2):
                            c0 = half * (TOK_BLK // 2)
                            psum = ps2.tile([128, TOK_BLK], mybir.dt.float32,
                                            tag="ps2", name=f"ps2_{t}_{d}_h{half}")
                            for k in range(K2):
                                nc.tensor.matmul(
                                    psum[:, :TOK_BLK // 2],
                                    w2_sb[:, d * dw + k * 128: d * dw + (k + 1) * 128],
                                    h_tiles[k][:, c0:c0 + TOK_BLK // 2],
                                    start=(k == 0), stop=(k == K2 - 1),
                                )
                            o_sb = opool.tile([128, TOK_BLK // 2], mybir.dt.float32,
                                              tag="o", name=f"o_sb{t}_{d}_h{half}")
                            nc.scalar.activation(o_sb[:], psum[:, :TOK_BLK // 2],
                                                 mybir.ActivationFunctionType.Identity,
                                                 bias=b2_sb[:, d:d + 1], scale=1.0)
                            nc.scalar.dma_start(
                                yT[d * 128:(d + 1) * 128,
                                   t * TOK_BLK + c0: t * TOK_BLK + c0 + TOK_BLK // 2],
                                o_sb[:])

    nc.compile()
    return nc


def _get_compiled():
    global _compiled
    if _compiled is None:
        _compiled = _build()
    return _compiled


def _make_in_maps(x, w1, b1, w2, b2):
    in_maps = []
    for e in range(E):
        xe = x[e * T_PER_E:(e + 1) * T_PER_E]            # [2048, 1024]
        xl = xe.reshape(N_TOK_BLK, TOK_BLK, K1, 128)     # t, c, k, p
        xl = xl.transpose(3, 0, 2, 1).reshape(128, -1)   # p, (t k c)
        w1e = w1[e].reshape(M1, 128, K1, 128)            # m, mc, k, p
        w1l = w1e.transpose(3, 0, 2, 1).reshape(128, -1)  # p, (m k mc)
        w2e = w2[e].reshape(M2, 128, K2, 128)            # d, dc, k, p
        w2l = w2e.transpose(3, 0, 2, 1).reshape(128, -1)  # p, (d k dc)
        in_maps.append({
            "xL": np.ascontiguousarray(xl).astype(NP_CDT),
            "w1L": np.ascontiguousarray(w1l).astype(NP_CDT),
            "w2L": np.ascontiguousarray(w2l).astype(NP_CDT),
            "b1r": np.ascontiguousarray(b1[e].reshape(M1, 128).T).astype(np.float32),
            "b2r": np.ascontiguousarray(b2[e].reshape(M2, 128).T).astype(np.float32),
        })
    return in_maps


def run(x, cnt, w1, b1, w2, b2, trace=False):
    nc = _get_compiled()
    in_maps = _make_in_maps(x, w1, b1, w2, b2)
    res = run_bass_kernel_spmd(nc, in_maps, core_ids=list(range(E)), trace=trace)
    outs = [res.results[e]["yT"].T for e in range(E)]
    y = np.concatenate(outs, axis=0).astype(np.float32)
    return y, res


def kernel(x, cnt, w1, b1, w2, b2):
    y, _ = run(x, cnt, w1, b1, w2, b2, trace=False)
    return y


# revision 5
# speedup vs baseline: 1.0065x; 1.0065x over previous
"""Expert-parallel MoE MLP (8 experts -> 8 NeuronCores) Bass kernel for TRN2.

Problem: y[t] = W2[e] @ gelu(W1[e] @ x[t] + b1[e]) + b2[e], tokens contiguous
per expert, 2048 tokens/expert, d_in=d_out=1024, d_hid=4096.

Sharding: expert-parallel. Core e gets expert e's weights and its 2048 tokens.
No cross-core communication needed (counts are equal and tokens are already
sorted by expert); host does the shard/unshard.

Per-core compute layout (everything partition-major, h kept as [hid, tok]):
  GEMM1: h[hid, tok]  = w1T[k,:].T @ xT[k, tok]   (accum over k = d_in tiles)
  GELU : h = gelu(h + b1)  via ScalarE with fused per-partition bias
  GEMM2: y[dout, tok] = w2T[k,:].T @ h[k, tok]    (accum over k = d_hid tiles)
  BIAS : y += b2 via ScalarE Identity with fused bias

Timing model (measured): ~7us framework preamble gates everything, first DMA
byte ~8.6us, steady-state MM spacing 215.6ns (= silicon floor for N=512 bf16).
So the only addressable time is the ramp-in (first-tile data latency), keeping
HAM warm across the ramp, and the serial tail after the last matmul.

v2 changes vs v1:
  - weights stream on the sync HWDGE ring, x blocks + y writebacks on the
    scalar HWDGE ring: the two physical rings drain in parallel, and the
    prologue-critical bytes (w1 m0 + x0 k0-1) are first on each ring.
  - fewer, larger DMA pieces (each dma_start costs ~0.6us dispatch on the
    issuing engine; v1's 8-way x0 split delayed w1's first byte by ~5us).
  - warmup matmuls sized to bridge PE activity until real data arrives
    (HAM stays at K=8/8; v1 had a 2.5us gap at ~14us that re-throttled
    the PE clock to 1.2GHz for 3.4us).
  - last output tile computed as two 256-column PSUM groups so the final
    activation+DMA overlaps the last 32 matmuls instead of serializing.
"""
import sys

sys.path.insert(0, "/opt/trn_rl_repo")

import numpy as np

import concourse.bass as bass  # noqa: F401
import concourse.tile as tile
from concourse import bacc, mybir
from concourse.bass_utils import run_bass_kernel_spmd

E = 8
T_PER_E = 2048
D_IN = 1024
D_HID = 4096
D_OUT = 1024

TOK_BLK = 512          # tokens per block (= PSUM bank free size in fp32)
N_TOK_BLK = T_PER_E // TOK_BLK
K1 = D_IN // 128       # k tiles for GEMM1
M1 = D_HID // 128      # output row tiles for GEMM1
K2 = D_HID // 128      # k tiles for GEMM2
M2 = D_OUT // 128      # output row tiles for GEMM2

CDT = mybir.dt.float16   # compute dtype on device (weights + activations)
NP_CDT = np.float16

N_WARM = 5               # N=512 warmup matmuls bridging until first data lands

_compiled = None


def _build():
    nc = bacc.Bacc("TRN2", target_bir_lowering=False, debug=False)

    # Host-permuted layouts (see _make_in_maps):
    #   xL [128, t*4096 + k*512 + c]   = x[t*512+c, k*128+p]
    #   w1L[128, m*1024 + k*128 + mc]  = w1[m*128+mc, k*128+p]
    #   w2L[128, d*4096 + k*128 + dc]  = w2[d*128+dc, k*128+p]
    xL = nc.dram_tensor("xL", [128, N_TOK_BLK * K1 * TOK_BLK], CDT, kind="ExternalInput").ap()
    w1L = nc.dram_tensor("w1L", [128, M1 * K1 * 128], CDT, kind="ExternalInput").ap()
    w2L = nc.dram_tensor("w2L", [128, M2 * K2 * 128], CDT, kind="ExternalInput").ap()
    b1r = nc.dram_tensor("b1r", [128, M1], mybir.dt.float32, kind="ExternalInput").ap()
    b2r = nc.dram_tensor("b2r", [128, M2], mybir.dt.float32, kind="ExternalInput").ap()
    yT = nc.dram_tensor("yT", [D_OUT, T_PER_E], mybir.dt.float32, kind="ExternalOutput").ap()

    XBLK = K1 * TOK_BLK  # 4096 cols per token block in xL
    mw = K1 * 128        # 1024 cols per w1 m-tile
    dw = K2 * 128        # 4096 cols per w2 d-tile

    with tile.TileContext(nc) as tc:
        with tc.tile_pool(name="wpool", bufs=1) as wpool, \
             tc.tile_pool(name="xpool", bufs=2) as xpool, \
             tc.tile_pool(name="hpool", bufs=1) as hpool, \
             tc.tile_pool(name="opool", bufs=4) as opool, \
             tc.tile_pool(name="ps1", bufs=3, space="PSUM") as ps1, \
             tc.tile_pool(name="ps2", bufs=4, space="PSUM") as ps2:

            # --- PE warmup: keep the HAM clock gate warm until data lands ---
            scr = wpool.tile([128, TOK_BLK], CDT, name="scr")
            nc.vector.memset(scr[:], 0.0)
            for i in range(N_WARM):
                wps = ps1.tile([128, TOK_BLK], mybir.dt.float32, tag="ps1", name=f"warm{i}")
                nc.tensor.matmul(wps[:], scr[:, :128], scr[:], start=True, stop=True)

            # --- prologue: the steady state needs x0 (1MB) + w1 m0/m1 (0.5MB)
            # resident first; split that critical set across BOTH HWDGE rings
            # so it drains at the full ~390GB/s HBM rate ---
            w1_sb = wpool.tile([128, M1 * mw], CDT, name="w1_sb")
            x_blocks = {}
            x_sb = xpool.tile([128, XBLK], CDT, tag="x", name="x_sb0")
            qx = XBLK // 4

            nc.sync.dma_start(w1_sb[:, 0:mw], w1L[:, 0:mw])                    # m0
            nc.scalar.dma_start(x_sb[:, 0:qx], xL[:, 0:qx])                    # x0 k01
            nc.sync.dma_start(w1_sb[:, mw:2 * mw], w1L[:, mw:2 * mw])          # m1
            nc.scalar.dma_start(x_sb[:, qx:2 * qx], xL[:, qx:2 * qx])          # x0 k23
            nc.sync.dma_start(x_sb[:, 2 * qx:3 * qx], xL[:, 2 * qx:3 * qx])    # x0 k45
            nc.scalar.dma_start(x_sb[:, 3 * qx:], xL[:, 3 * qx:XBLK])          # x0 k67
            x_blocks[0] = x_sb

            b1_sb = wpool.tile([128, M1], mybir.dt.float32, name="b1_sb")
            nc.sync.dma_start(b1_sb[:], b1r[:, :])
            b2_sb = wpool.tile([128, M2], mybir.dt.float32, name="b2_sb")
            nc.sync.dma_start(b2_sb[:], b2r[:, :])

            for m in range(2, M1):
                nc.sync.dma_start(w1_sb[:, m * mw:(m + 1) * mw],
                                  w1L[:, m * mw:(m + 1) * mw])

            x_sb1 = xpool.tile([128, XBLK], CDT, tag="x", name="x_sb1")
            for j in range(2):
                w = XBLK // 2
                nc.scalar.dma_start(x_sb1[:, j * w:(j + 1) * w],
                                    xL[:, XBLK + j * w: XBLK + (j + 1) * w])
            x_blocks[1] = x_sb1

            # --- sync ring: w2, one d-tile per piece ---
            w2_sb = wpool.tile([128, M2 * dw], CDT, name="w2_sb")
            for d in range(M2):
                nc.sync.dma_start(w2_sb[:, d * dw:(d + 1) * dw],
                                  w2L[:, d * dw:(d + 1) * dw])

            for t in range(N_TOK_BLK):
                if t in x_blocks:
                    x_sb = x_blocks[t]
                else:
                    x_sb = xpool.tile([128, XBLK], CDT, tag="x", name=f"x_sb{t}")
                    for j in range(2):
                        w = XBLK // 2
                        nc.scalar.dma_start(x_sb[:, j * w:(j + 1) * w],
                                            xL[:, t * XBLK + j * w: t * XBLK + (j + 1) * w])

                # --- GEMM1 + gelu: h[m] tiles ---
                h_tiles = []
                for m in range(M1):
                    psum = ps1.tile([128, TOK_BLK], mybir.dt.float32,
                                    tag="ps1", name=f"ps1_{t}_{m}")
                    for k in range(K1):
                        nc.tensor.matmul(
                            psum[:],
                            w1_sb[:, m * mw + k * 128: m * mw + (k + 1) * 128],
                            x_sb[:, k * TOK_BLK:(k + 1) * TOK_BLK],
                            start=(k == 0), stop=(k == K1 - 1),
                        )
                    h_sb = hpool.tile([128, TOK_BLK], CDT, tag=f"h{m}",
                                      name=f"h_sb{t}_{m}")
                    nc.scalar.activation(h_sb[:], psum[:],
                                         mybir.ActivationFunctionType.Gelu,
                                         bias=b1_sb[:, m:m + 1], scale=1.0)
                    h_tiles.append(h_sb)

                # --- GEMM2 + bias: y[d] tiles ---
                for d in range(M2):
                    last = (t == N_TOK_BLK - 1 and d == M2 - 1)
                    if not last:
                        psum = ps2.tile([128, TOK_BLK], mybir.dt.float32,
                                        tag="ps2", name=f"ps2_{t}_{d}")
                        for k in range(K2):
                            nc.tensor.matmul(
                                psum[:],
                                w2_sb[:, d * dw + k * 128: d * dw + (k + 1) * 128],
                                h_tiles[k][:],
                                start=(k == 0), stop=(k == K2 - 1),
                            )
                        o_sb = opool.tile([128, TOK_BLK], mybir.dt.float32,
                                          tag="o", name=f"o_sb{t}_{d}")
                        nc.scalar.activation(o_sb[:], psum[:],
                                             mybir.ActivationFunctionType.Identity,
                                             bias=b2_sb[:, d:d + 1], scale=1.0)
                        nc.scalar.dma_start(yT[d * 128:(d + 1) * 128,
                                               t * TOK_BLK:(t + 1) * TOK_BLK],
                                            o_sb[:])
                    else:
                        # final tile: two half-column groups so the last
                        # activation+DMA overlaps the second group's matmuls
                        for half in range(2):
                            c0 = half * (TOK_BLK // 2)
                            psum = ps2.tile([128, TOK_BLK], mybir.dt.float32,
                                            tag="ps2", name=f"ps2_{t}_{d}_h{half}")
                            for k in range(K2):
                                nc.tensor.matmul(
                                    psum[:, :TOK_BLK // 2],
                                    w2_sb[:, d * dw + k * 128: d * dw + (k + 1) * 128],
                                    h_tiles[k][:, c0:c0 + TOK_BLK // 2],
                                    start=(k == 0), stop=(k == K2 - 1),
                                )
                            o_sb = opool.tile([128, TOK_BLK // 2], mybir.dt.float32,
                                              tag="o", name=f"o_sb{t}_{d}_h{half}")
                            nc.scalar.activation(o_sb[:], psum[:, :TOK_BLK // 2],
                                                 mybir.ActivationFunctionType.Identity,
                                                 bias=b2_sb[:, d:d + 1], scale=1.0)
                            # split the final flight across both rings
                            q = TOK_BLK // 4
                            nc.scalar.dma_start(
                                yT[d * 128:(d + 1) * 128,
                                   t * TOK_BLK + c0: t * TOK_BLK + c0 + q],
                                o_sb[:, :q])
                            nc.sync.dma_start(
                                yT[d * 128:(d + 1) * 128,
                                   t * TOK_BLK + c0 + q: t * TOK_BLK + c0 + 2 * q],
                                o_sb[:, q:])

    nc.compile()
    return nc


def _get_compiled():
    global _compiled
    if _compiled is None:
        _compiled = _build()
    return _compiled


def _make_in_maps(x, w1, b1, w2, b2):
    in_maps = []
    for e in range(E):
        xe = x[e * T_PER_E:(e + 1) * T_PER_E]            # [2048, 1024]
        xl = xe.reshape(N_TOK_BLK, TOK_BLK, K1, 128)     # t, c, k, p
        xl = xl.transpose(3, 0, 2, 1).reshape(128, -1)   # p, (t k c)
        w1e = w1[e].reshape(M1, 128, K1, 128)            # m, mc, k, p
        w1l = w1e.transpose(3, 0, 2, 1).reshape(128, -1)  # p, (m k mc)
        w2e = w2[e].reshape(M2, 128, K2, 128)            # d, dc, k, p
        w2l = w2e.transpose(3, 0, 2, 1).reshape(128, -1)  # p, (d k dc)
        in_maps.append({
            "xL": np.ascontiguousarray(xl).astype(NP_CDT),
            "w1L": np.ascontiguousarray(w1l).astype(NP_CDT),
            "w2L": np.ascontiguousarray(w2l).astype(NP_CDT),
            "b1r": np.ascontiguousarray(b1[e].reshape(M1, 128).T).astype(np.float32),
            "b2r": np.ascontiguousarray(b2[e].reshape(M2, 128).T).astype(np.float32),
        })
    return in_maps


def run(x, cnt, w1, b1, w2, b2, trace=False):
    nc = _get_compiled()
    in_maps = _make_in_maps(x, w1, b1, w2, b2)
    res = run_bass_kernel_spmd(nc, in_maps, core_ids=list(range(E)), trace=trace)
    outs = [res.results[e]["yT"].T for e in range(E)]
    y = np.concatenate(outs, axis=0).astype(np.float32)
    return y, res


def kernel(x, cnt, w1, b1, w2, b2):
    y, _ = run(x, cnt, w1, b1, w2, b2, trace=False)
    return y


# revision 7
# speedup vs baseline: 1.0313x; 1.0246x over previous
"""Expert-parallel MoE MLP (8 experts -> 8 NeuronCores) Bass kernel for TRN2.

Problem: y[t] = W2[e] @ gelu(W1[e] @ x[t] + b1[e]) + b2[e], tokens contiguous
per expert, 2048 tokens/expert, d_in=d_out=1024, d_hid=4096.

Sharding: expert-parallel. Core e gets expert e's weights and its 2048 tokens.
No cross-core communication needed (counts are equal and tokens are already
sorted by expert); host does the shard/unshard.

Per-core compute layout (everything partition-major, h kept as [hid, tok]):
  GEMM1: h[hid, tok]  = w1T[k,:].T @ xT[k, tok]   (accum over k = d_in tiles)
  GELU : h = gelu(h/512 + b1)  via ScalarE with fused per-partition bias
  GEMM2: y[dout, tok] = w2T[k,:].T @ h[k, tok]    (accum over k = d_hid tiles)
  BIAS : y += b2 via ScalarE Identity with fused bias

Measured structure: ~7us framework preamble gates everything; steady-state
fp16 MM spacing is 215.6ns (silicon floor for N=512); so the levers are the
ramp-in, HAM warmth, the serial tail, and shaving PE-cycles via fp8.

Fractional fp8 (DoubleRow): for the first NS_FP8 of 32 GEMM1 m-tiles, the
d_in 0..255 contraction slice runs as ONE fp8 DoubleRow matmul (2 MACs/cell/
cycle) instead of two fp16 matmuls. Operands are quantized on the HOST
(fp8(4x), fp8(128*w1), e4m3), accumulated in fp32 PSUM; the fp16 k-slices use
w1 pre-scaled by 512 (exact power-of-2 in fp16), so the whole PSUM carries a
uniform x512 factor that the gelu evacuation removes via scale=1/512. Zero
extra on-chip ops. Offline-simulated max-rel error vs the fp32 reference:
1.41e-2 at NS_FP8=28 (gate: 2e-2); the fp8 values are host-generated so the
simulation is exact up to fp32 accumulation order.
"""
import sys

sys.path.insert(0, "/opt/trn_rl_repo")

import numpy as np
import ml_dtypes

import concourse.bass as bass  # noqa: F401
import concourse.tile as tile
from concourse import bacc, mybir
from concourse.bass_utils import run_bass_kernel_spmd

E = 8
T_PER_E = 2048
D_IN = 1024
D_HID = 4096
D_OUT = 1024

TOK_BLK = 512          # tokens per block (= PSUM bank free size in fp32)
N_TOK_BLK = T_PER_E // TOK_BLK
K1 = D_IN // 128       # k tiles for GEMM1
M1 = D_HID // 128      # output row tiles for GEMM1
K2 = D_HID // 128      # k tiles for GEMM2
M2 = D_OUT // 128      # output row tiles for GEMM2

CDT = mybir.dt.float16   # compute dtype on device (weights + activations)
NP_CDT = np.float16
F8 = mybir.dt.float8e4
NP_F8 = ml_dtypes.float8_e4m3

NS_FP8 = 28              # m-tiles whose k-pair {0,1} runs as one fp8 DR matmul
W1_SCALE = 512.0         # fp16 w1 pre-scale (psum carries x512, gelu divides)
X8_SCALE = 4.0           # fp8 x pre-scale   (4 * 128 = 512)
W8_SCALE = 128.0         # fp8 w1 pre-scale

N_WARM = 12              # N=512 warmup matmuls bridging until first data lands

_compiled = None


def _build():
    nc = bacc.Bacc("TRN2", target_bir_lowering=False, debug=False)

    # Host-permuted layouts (see _make_in_maps):
    #   xL [128, t*4096 + k*512 + c]   = x[t*512+c, k*128+p] * 1      (fp16)
    #   xP8[128, t*1024 + i*512 + c]   = x[t*512+c, i*128+p] * 4      (fp8)
    #   w1L[128, m*1024 + k*128 + mc]  = w1[m*128+mc, k*128+p] * 512  (fp16)
    #   w1P8[128, m*256 + i*128 + mc]  = w1[m*128+mc, i*128+p] * 128  (fp8)
    #   w2L[128, d*4096 + k*128 + dc]  = w2[d*128+dc, k*128+p]        (fp16)
    xL = nc.dram_tensor("xL", [128, N_TOK_BLK * K1 * TOK_BLK], CDT, kind="ExternalInput").ap()
    xP8 = nc.dram_tensor("xP8", [128, N_TOK_BLK * 2 * TOK_BLK], F8, kind="ExternalInput").ap()
    w1L = nc.dram_tensor("w1L", [128, M1 * K1 * 128], CDT, kind="ExternalInput").ap()
    w1P8 = nc.dram_tensor("w1P8", [128, NS_FP8 * 256], F8, kind="ExternalInput").ap()
    w2L = nc.dram_tensor("w2L", [128, M2 * K2 * 128], CDT, kind="ExternalInput").ap()
    b1r = nc.dram_tensor("b1r", [128, M1], mybir.dt.float32, kind="ExternalInput").ap()
    b2r = nc.dram_tensor("b2r", [128, M2], mybir.dt.float32, kind="ExternalInput").ap()
    yT = nc.dram_tensor("yT", [D_OUT, T_PER_E], mybir.dt.float32, kind="ExternalOutput").ap()

    XBLK = K1 * TOK_BLK   # 4096 fp16 cols per token block in xL
    X8BLK = 2 * TOK_BLK   # 1024 fp8 cols per token block in xP8
    mw = K1 * 128         # 1024 cols per w1 m-tile
    dw = K2 * 128         # 4096 cols per w2 d-tile

    with tile.TileContext(nc) as tc:
        with tc.tile_pool(name="wpool", bufs=1) as wpool, \
             tc.tile_pool(name="xpool", bufs=2) as xpool, \
             tc.tile_pool(name="x8pool", bufs=2) as x8pool, \
             tc.tile_pool(name="hpool", bufs=1) as hpool, \
             tc.tile_pool(name="opool", bufs=4) as opool, \
             tc.tile_pool(name="ps1", bufs=3, space="PSUM") as ps1, \
             tc.tile_pool(name="ps2", bufs=4, space="PSUM") as ps2:

            # --- PE warmup: keep the HAM clock gate warm until data lands ---
            scr = wpool.tile([128, TOK_BLK], CDT, name="scr")
            nc.vector.memset(scr[:], 0.0)
            for i in range(N_WARM):
                wps = ps1.tile([128, TOK_BLK], mybir.dt.float32, tag="ps1", name=f"warm{i}")
                nc.tensor.matmul(wps[:], scr[:, :128], scr[:], start=True, stop=True)

            # --- prologue: critical set = xP8 t0 + x0 k2..7 + w1P8 m0-3 +
            # w1 m0/m1; split across BOTH HWDGE rings (sync + scalar) ---
            w1_sb = wpool.tile([128, M1 * mw], CDT, name="w1_sb")
            w1p8_sb = wpool.tile([128, NS_FP8 * 256], F8, name="w1p8_sb")
            x_blocks = {}
            x8_blocks = {}
            x_sb = xpool.tile([128, XBLK], CDT, tag="x", name="x_sb0")
            x8_sb = x8pool.tile([128, X8BLK], F8, tag="x8", name="x8_sb0")
            qx = XBLK // 4

            nc.sync.dma_start(w1p8_sb[:, 0:1024], w1P8[:, 0:1024])             # fp8 m0-3
            nc.scalar.dma_start(x8_sb[:], xP8[:, 0:X8BLK])                     # fp8 x t0
            nc.sync.dma_start(w1_sb[:, 0:mw], w1L[:, 0:mw])                    # m0
            nc.scalar.dma_start(x_sb[:, qx:2 * qx], xL[:, qx:2 * qx])          # x0 k23
            nc.sync.dma_start(w1_sb[:, mw:2 * mw], w1L[:, mw:2 * mw])          # m1
            nc.scalar.dma_start(x_sb[:, 2 * qx:3 * qx], xL[:, 2 * qx:3 * qx])  # x0 k45
            nc.sync.dma_start(w1p8_sb[:, 1024:], w1P8[:, 1024:])               # fp8 m4+
            nc.scalar.dma_start(x_sb[:, 3 * qx:], xL[:, 3 * qx:XBLK])          # x0 k67
            x_blocks[0] = x_sb
            x8_blocks[0] = x8_sb

            b1_sb = wpool.tile([128, M1], mybir.dt.float32, name="b1_sb")
            nc.sync.dma_start(b1_sb[:], b1r[:, :])
            b2_sb = wpool.tile([128, M2], mybir.dt.float32, name="b2_sb")
            nc.sync.dma_start(b2_sb[:], b2r[:, :])

            for m in range(2, M1):
                nc.sync.dma_start(w1_sb[:, m * mw:(m + 1) * mw],
                                  w1L[:, m * mw:(m + 1) * mw])

            x8_sb1 = x8pool.tile([128, X8BLK], F8, tag="x8", name="x8_sb1")
            nc.scalar.dma_start(x8_sb1[:], xP8[:, X8BLK:2 * X8BLK])
            x8_blocks[1] = x8_sb1
            x_sb1 = xpool.tile([128, XBLK], CDT, tag="x", name="x_sb1")
            for j in range(2):
                w = XBLK // 2
                nc.scalar.dma_start(x_sb1[:, j * w:(j + 1) * w],
                                    xL[:, XBLK + j * w: XBLK + (j + 1) * w])
            x_blocks[1] = x_sb1
            # x0 k01 fp16: only the unscoped m-tiles (m >= NS_FP8) need it,
            # and they run ~50us into block 0 -> lowest priority
            nc.scalar.dma_start(x_sb[:, 0:qx], xL[:, 0:qx])

            # --- sync ring: w2, one d-tile per piece ---
            w2_sb = wpool.tile([128, M2 * dw], CDT, name="w2_sb")
            for d in range(M2):
                nc.sync.dma_start(w2_sb[:, d * dw:(d + 1) * dw],
                                  w2L[:, d * dw:(d + 1) * dw])

            for t in range(N_TOK_BLK):
                if t in x_blocks:
                    x_sb = x_blocks[t]
                    x8_sb = x8_blocks[t]
                else:
                    x8_sb = x8pool.tile([128, X8BLK], F8, tag="x8", name=f"x8_sb{t}")
                    nc.scalar.dma_start(x8_sb[:], xP8[:, t * X8BLK:(t + 1) * X8BLK])
                    x_sb = xpool.tile([128, XBLK], CDT, tag="x", name=f"x_sb{t}")
                    for j in range(2):
                        w = XBLK // 2
                        nc.scalar.dma_start(x_sb[:, j * w:(j + 1) * w],
                                            xL[:, t * XBLK + j * w: t * XBLK + (j + 1) * w])

                x8_ap = x8_sb[:, :].rearrange("p (i n) -> p i n", i=2)

                # --- GEMM1 + gelu: h[m] tiles ---
                h_tiles = []
                for m in range(M1):
                    psum = ps1.tile([128, TOK_BLK], mybir.dt.float32,
                                    tag="ps1", name=f"ps1_{t}_{m}")
                    if m < NS_FP8:
                        w8_ap = w1p8_sb[:, m * 256:(m + 1) * 256].rearrange(
                            "p (i c) -> p i c", i=2)
                        nc.tensor.matmul(psum[:], w8_ap, x8_ap,
                                         start=True, stop=False,
                                         perf_mode=mybir.MatmulPerfMode.DoubleRow)
                        k_lo = 2
                    else:
                        k_lo = 0
                    for k in range(k_lo, K1):
                        nc.tensor.matmul(
                            psum[:],
                            w1_sb[:, m * mw + k * 128: m * mw + (k + 1) * 128],
                            x_sb[:, k * TOK_BLK:(k + 1) * TOK_BLK],
                            start=(k == 0 and k_lo == 0), stop=(k == K1 - 1),
                        )
                    h_sb = hpool.tile([128, TOK_BLK], CDT, tag=f"h{m}",
                                      name=f"h_sb{t}_{m}")
                    nc.scalar.activation(h_sb[:], psum[:],
                                         mybir.ActivationFunctionType.Gelu,
                                         bias=b1_sb[:, m:m + 1], scale=1.0 / W1_SCALE)
                    h_tiles.append(h_sb)

                # --- GEMM2 + bias: y[d] tiles ---
                for d in range(M2):
                    last = (t == N_TOK_BLK - 1 and d == M2 - 1)
                    if not last:
                        psum = ps2.tile([128, TOK_BLK], mybir.dt.float32,
                                        tag="ps2", name=f"ps2_{t}_{d}")
                        for k in range(K2):
                            nc.tensor.matmul(
                                psum[:],
                                w2_sb[:, d * dw + k * 128: d * dw + (k + 1) * 128],
                                h_tiles[k][:],
                                start=(k == 0), stop=(k == K2 - 1),
                            )
                        o_sb = opool.tile([128, TOK_BLK], mybir.dt.float32,
                                          tag="o", name=f"o_sb{t}_{d}")
                        nc.scalar.activation(o_sb[:], psum[:],
                                             mybir.ActivationFunctionType.Identity,
                                             bias=b2_sb[:, d:d + 1], scale=1.0)
                        nc.scalar.dma_start(yT[d * 128:(d + 1) * 128,
                                               t * TOK_BLK:(t + 1) * TOK_BLK],
                                            o_sb[:])
                    else:
                        # final tile: two half-column groups so the last
                        # activation+DMA overlaps the second group's matmuls
                        for half in range(2):
                            c0 = half * (TOK_BLK // 2)
                            psum = ps2.tile([128, TOK_BLK], mybir.dt.float32,
                                            tag="ps2", name=f"ps2_{t}_{d}_h{half}")
                            for k in range(K2):
                                nc.tensor.matmul(
                                    psum[:, :TOK_BLK // 2],
                                    w2_sb[:, d * dw + k * 128: d * dw + (k + 1) * 128],
                                    h_tiles[k][:, c0:c0 + TOK_BLK // 2],
                                    start=(k == 0), stop=(k == K2 - 1),
                                )
                            o_sb = opool.tile([128, TOK_BLK // 2], mybir.dt.float32,
                                              tag="o", name=f"o_sb{t}_{d}_h{half}")
                            nc.scalar.activation(o_sb[:], psum[:, :TOK_BLK // 2],
                                                 mybir.ActivationFunctionType.Identity,
                                                 bias=b2_sb[:, d:d + 1], scale=1.0)
                            # split the final flight across both rings
                            q = TOK_BLK // 4
                            nc.scalar.dma_start(
                                yT[d * 128:(d + 1) * 128,
                                   t * TOK_BLK + c0: t * TOK_BLK + c0 + q],
                                o_sb[:, :q])
                            nc.sync.dma_start(
                                yT[d * 128:(d + 1) * 128,
                                   t * TOK_BLK + c0 + q: t * TOK_BLK + c0 + 2 * q],
                                o_sb[:, q:])

    nc.compile()
    return nc


def _get_compiled():
    global _compiled
    if _compiled is None:
        _compiled = _build()
    return _compiled


def _q8(a):
    return np.clip(a, -240.0, 240.0).astype(NP_F8)


def _make_in_maps(x, w1, b1, w2, b2):
    in_maps = []
    for e in range(E):
        xe = x[e * T_PER_E:(e + 1) * T_PER_E].astype(np.float32)  # [2048, 1024]
        xl = xe.reshape(N_TOK_BLK, TOK_BLK, K1, 128)     # t, c, k, p
        xl = xl.transpose(3, 0, 2, 1).reshape(128, -1)   # p, (t k c)
        # xP8 covers d_in 0..255 only: columns [t, i, c]
        x8 = xe[:, :256].reshape(N_TOK_BLK, TOK_BLK, 2, 128)
        x8 = x8.transpose(3, 0, 2, 1).reshape(128, -1)   # p, (t i c)
        w1e = (w1[e].astype(np.float32) * W1_SCALE).reshape(M1, 128, K1, 128)
        w1l = w1e.transpose(3, 0, 2, 1).reshape(128, -1)  # p, (m k mc)
        w1p = (w1[e][:NS_FP8 * 128, :256].astype(np.float32) * W8_SCALE)
        w1p = w1p.reshape(NS_FP8, 128, 2, 128)           # m, mc, i, p
        w1p = w1p.transpose(3, 0, 2, 1).reshape(128, -1)  # p, (m i mc)
        w2e = w2[e].reshape(M2, 128, K2, 128)            # d, dc, k, p
        w2l = w2e.transpose(3, 0, 2, 1).reshape(128, -1)  # p, (d k dc)
        in_maps.append({
            "xL": np.ascontiguousarray(xl).astype(NP_CDT),
            "xP8": _q8(np.ascontiguousarray(x8) * X8_SCALE),
            "w1L": np.ascontiguousarray(w1l).astype(NP_CDT),
            "w1P8": _q8(np.ascontiguousarray(w1p)),
            "w2L": np.ascontiguousarray(w2l).astype(NP_CDT),
            "b1r": np.ascontiguousarray(b1[e].reshape(M1, 128).T).astype(np.float32),
            "b2r": np.ascontiguousarray(b2[e].reshape(M2, 128).T).astype(np.float32),
        })
    return in_maps


def run(x, cnt, w1, b1, w2, b2, trace=False):
    nc = _get_compiled()
    in_maps = _make_in_maps(x, w1, b1, w2, b2)
    res = run_bass_kernel_spmd(nc, in_maps, core_ids=list(range(E)), trace=trace)
    outs = [res.results[e]["yT"].T for e in range(E)]
    y = np.concatenate(outs, axis=0).astype(np.float32)
    return y, res


def kernel(x, cnt, w1, b1, w2, b2):
    y, _ = run(x, cnt, w1, b1, w2, b2, trace=False)
    return y


# revision 11
# speedup vs baseline: 1.0436x; 1.0120x over previous
"""Expert-parallel MoE MLP (8 experts -> 8 NeuronCores) Bass kernel for TRN2.

Problem: y[t] = W2[e] @ gelu(W1[e] @ x[t] + b1[e]) + b2[e], tokens contiguous
per expert, 2048 tokens/expert, d_in=d_out=1024, d_hid=4096.

Sharding: expert-parallel. Core e gets expert e's weights and its 2048 tokens.
No cross-core communication needed (counts are equal and tokens are already
sorted by expert); host does the shard/unshard.

Per-core compute layout (everything partition-major, h kept as [hid, tok]):
  GEMM1: h[hid, tok]  = w1T[k,:].T @ xT[k, tok]   (accum over k = d_in tiles)
  GELU : h = gelu(h/512 + b1)  via ScalarE with fused per-partition bias
  GEMM2: y[dout, tok] = w2T[k,:].T @ h[k, tok]    (accum over k = d_hid tiles)
  BIAS : y += b2 via ScalarE Identity with fused bias

Measured structure: ~7us framework preamble gates everything; steady-state
fp16 MM spacing is 215.6ns (silicon floor for N=512); so the levers are the
ramp-in, HAM warmth, the serial tail, and shaving PE-cycles via fp8.

Fractional fp8 (DoubleRow): for the first NS_FP8 of 32 GEMM1 m-tiles in
token blocks 1-3 (block 0 stays fp16: DR + the concurrent ~300GB/s weight
prefetch trips the clock throttler), the d_in 0..255 contraction slice runs
as ONE fp8 DoubleRow matmul (2 MACs/cell/cycle) instead of two fp16 matmuls. Operands are quantized on the HOST
(fp8(4x), fp8(128*w1), e4m3), accumulated in fp32 PSUM; the fp16 k-slices use
w1 pre-scaled by 512 (exact power-of-2 in fp16), so the whole PSUM carries a
uniform x512 factor that the gelu evacuation removes via scale=1/512. Zero
extra on-chip ops. Offline-simulated max-rel error vs the fp32 reference:
1.41e-2 at NS_FP8=28 (gate: 2e-2); the fp8 values are host-generated so the
simulation is exact up to fp32 accumulation order.
"""
import sys

sys.path.insert(0, "/opt/trn_rl_repo")

import numpy as np
import ml_dtypes

import concourse.bass as bass  # noqa: F401
import concourse.tile as tile
from concourse import bacc, mybir
from concourse.bass_utils import run_bass_kernel_spmd

E = 8
T_PER_E = 2048
D_IN = 1024
D_HID = 4096
D_OUT = 1024

TOK_BLK = 512          # tokens per block (= PSUM bank free size in fp32)
N_TOK_BLK = T_PER_E // TOK_BLK
K1 = D_IN // 128       # k tiles for GEMM1
M1 = D_HID // 128      # output row tiles for GEMM1
K2 = D_HID // 128      # k tiles for GEMM2
M2 = D_OUT // 128      # output row tiles for GEMM2

CDT = mybir.dt.float16   # compute dtype on device (weights + activations)
NP_CDT = np.float16
F8 = mybir.dt.float8e4
NP_F8 = ml_dtypes.float8_e4m3

NS_FP8 = 28              # m-tiles whose k-pair {0,1} runs as one fp8 DR matmul
W1_SCALE = 512.0         # fp16 w1 pre-scale (psum carries x512, gelu divides)
X8_SCALE = 4.0           # fp8 x pre-scale   (4 * 128 = 512)
W8_SCALE = 128.0         # fp8 w1 pre-scale

N_WARM = 12              # N=512 warmup matmuls bridging until first data lands

_compiled = None


def _build():
    nc = bacc.Bacc("TRN2", target_bir_lowering=False, debug=False)

    # Host-permuted layouts (see _make_in_maps):
    #   xL [128, t*4096 + k*512 + c]   = x[t*512+c, k*128+p] * 1      (fp16)
    #   xP8[128, t*1024 + i*512 + c]   = x[t*512+c, i*128+p] * 4      (fp8)
    #   w1L[128, m*1024 + k*128 + mc]  = w1[m*128+mc, k*128+p] * 512  (fp16)
    #   w1P8[128, m*256 + i*128 + mc]  = w1[m*128+mc, i*128+p] * 128  (fp8)
    #   w2L[128, d*4096 + k*128 + dc]  = w2[d*128+dc, k*128+p]        (fp16)
    xL = nc.dram_tensor("xL", [128, N_TOK_BLK * K1 * TOK_BLK], CDT, kind="ExternalInput").ap()
    xP8 = nc.dram_tensor("xP8", [128, N_TOK_BLK * 2 * TOK_BLK], F8, kind="ExternalInput").ap()
    w1L = nc.dram_tensor("w1L", [128, M1 * K1 * 128], CDT, kind="ExternalInput").ap()
    w1P8 = nc.dram_tensor("w1P8", [128, NS_FP8 * 256], F8, kind="ExternalInput").ap()
    w2L = nc.dram_tensor("w2L", [128, M2 * K2 * 128], CDT, kind="ExternalInput").ap()
    b1r = nc.dram_tensor("b1r", [128, M1], mybir.dt.float32, kind="ExternalInput").ap()
    b2r = nc.dram_tensor("b2r", [128, M2], mybir.dt.float32, kind="ExternalInput").ap()
    yT = nc.dram_tensor("yT", [D_OUT, T_PER_E], mybir.dt.float32, kind="ExternalOutput").ap()

    XBLK = K1 * TOK_BLK   # 4096 fp16 cols per token block in xL
    X8BLK = 2 * TOK_BLK   # 1024 fp8 cols per token block in xP8
    mw = K1 * 128         # 1024 cols per w1 m-tile
    dw = K2 * 128         # 4096 cols per w2 d-tile

    with tile.TileContext(nc) as tc:
        with tc.tile_pool(name="wpool", bufs=1) as wpool, \
             tc.tile_pool(name="xpool", bufs=2) as xpool, \
             tc.tile_pool(name="x8pool", bufs=2) as x8pool, \
             tc.tile_pool(name="hpool", bufs=1) as hpool, \
             tc.tile_pool(name="opool", bufs=4) as opool, \
             tc.tile_pool(name="ps1", bufs=3, space="PSUM") as ps1, \
             tc.tile_pool(name="ps2", bufs=4, space="PSUM") as ps2:

            # --- PE warmup: keep the HAM clock gate warm until data lands ---
            scr = wpool.tile([128, TOK_BLK], CDT, name="scr")
            nc.vector.memset(scr[:], 0.0)
            for i in range(N_WARM):
                wps = ps1.tile([128, TOK_BLK], mybir.dt.float32, tag="ps1", name=f"warm{i}")
                nc.tensor.matmul(wps[:], scr[:, :128], scr[:], start=True, stop=True)

            # --- prologue: block 0 runs pure fp16 (the fp8 DR matmuls are
            # scoped to blocks 1-3: DR + the concurrent ~300GB/s weight
            # stream trips the clock throttler; blocks 1-3 have light DMA).
            # Critical set = x0 (1MB) + w1 m0/m1, split across BOTH HWDGE
            # rings (sync + scalar) so it drains at full HBM rate ---
            w1_sb = wpool.tile([128, M1 * mw], CDT, name="w1_sb")
            w1p8_sb = wpool.tile([128, NS_FP8 * 256], F8, name="w1p8_sb")
            x_blocks = {}
            x8_blocks = {}
            x_sb = xpool.tile([128, XBLK], CDT, tag="x", name="x_sb0")
            qx = XBLK // 4

            nc.sync.dma_start(w1_sb[:, 0:mw], w1L[:, 0:mw])                    # m0
            nc.scalar.dma_start(x_sb[:, 0:qx], xL[:, 0:qx])                    # x0 k01
            nc.sync.dma_start(w1_sb[:, mw:2 * mw], w1L[:, mw:2 * mw])          # m1
            nc.scalar.dma_start(x_sb[:, qx:2 * qx], xL[:, qx:2 * qx])          # x0 k23
            nc.sync.dma_start(x_sb[:, 2 * qx:3 * qx], xL[:, 2 * qx:3 * qx])    # x0 k45
            nc.scalar.dma_start(x_sb[:, 3 * qx:], xL[:, 3 * qx:XBLK])          # x0 k67
            x_blocks[0] = x_sb

            b1_sb = wpool.tile([128, M1], mybir.dt.float32, name="b1_sb")
            nc.sync.dma_start(b1_sb[:], b1r[:, :])
            b2_sb = wpool.tile([128, M2], mybir.dt.float32, name="b2_sb")
            nc.sync.dma_start(b2_sb[:], b2r[:, :])

            for m in range(2, M1):
                nc.sync.dma_start(w1_sb[:, m * mw:(m + 1) * mw],
                                  w1L[:, m * mw:(m + 1) * mw])

            x_sb1 = xpool.tile([128, XBLK], CDT, tag="x", name="x_sb1")
            for j in range(2):
                w = XBLK // 2
                nc.scalar.dma_start(x_sb1[:, j * w:(j + 1) * w],
                                    xL[:, XBLK + j * w: XBLK + (j + 1) * w])
            x_blocks[1] = x_sb1
            x8_sb1 = x8pool.tile([128, X8BLK], F8, tag="x8", name="x8_sb1")
            nc.scalar.dma_start(x8_sb1[:], xP8[:, X8BLK:2 * X8BLK])
            x8_blocks[1] = x8_sb1

            # --- sync ring: w2 (one d-tile per piece), then the fp8 w1
            # (first needed at G1(t=1), ~125us in) ---
            w2_sb = wpool.tile([128, M2 * dw], CDT, name="w2_sb")
            for d in range(M2):
                nc.sync.dma_start(w2_sb[:, d * dw:(d + 1) * dw],
                                  w2L[:, d * dw:(d + 1) * dw])
            nc.sync.dma_start(w1p8_sb[:], w1P8[:, :])

            for t in range(N_TOK_BLK):
                if t in x_blocks:
                    x_sb = x_blocks[t]
                else:
                    x8_sb = x8pool.tile([128, X8BLK], F8, tag="x8", name=f"x8_sb{t}")
                    nc.scalar.dma_start(x8_sb[:], xP8[:, t * X8BLK:(t + 1) * X8BLK])
                    x8_blocks[t] = x8_sb
                    x_sb = xpool.tile([128, XBLK], CDT, tag="x", name=f"x_sb{t}")
                    for j in range(2):
                        w = XBLK // 2
                        nc.scalar.dma_start(x_sb[:, j * w:(j + 1) * w],
                                            xL[:, t * XBLK + j * w: t * XBLK + (j + 1) * w])

                use_dr = t >= 1
                if use_dr:
                    x8_ap = x8_blocks[t][:, :].rearrange("p (i n) -> p i n", i=2)

                # --- GEMM1 + gelu: h[m] tiles ---
                h_tiles = []
                for m in range(M1):
                    psum = ps1.tile([128, TOK_BLK], mybir.dt.float32,
                                    tag="ps1", name=f"ps1_{t}_{m}")
                    if use_dr and m < NS_FP8:
                        w8_ap = w1p8_sb[:, m * 256:(m + 1) * 256].rearrange(
                            "p (i c) -> p i c", i=2)
                        nc.tensor.matmul(psum[:], w8_ap, x8_ap,
                                         start=True, stop=False,
                                         perf_mode=mybir.MatmulPerfMode.DoubleRow)
                        k_lo = 2
                    else:
                        k_lo = 0
                    for k in range(k_lo, K1):
                        nc.tensor.matmul(
                            psum[:],
                            w1_sb[:, m * mw + k * 128: m * mw + (k + 1) * 128],
                            x_sb[:, k * TOK_BLK:(k + 1) * TOK_BLK],
                            start=(k == 0 and k_lo == 0), stop=(k == K1 - 1),
                        )
                    h_sb = hpool.tile([128, TOK_BLK], CDT, tag=f"h{m}",
                                      name=f"h_sb{t}_{m}")
                    nc.scalar.activation(h_sb[:], psum[:],
                                         mybir.ActivationFunctionType.Gelu,
                                         bias=b1_sb[:, m:m + 1], scale=1.0 / W1_SCALE)
                    h_tiles.append(h_sb)

                # --- GEMM2 + bias: y[d] tiles ---
                for d in range(M2):
                    last = (t == N_TOK_BLK - 1 and d == M2 - 1)
                    if not last:
                        psum = ps2.tile([128, TOK_BLK], mybir.dt.float32,
                                        tag="ps2", name=f"ps2_{t}_{d}")
                        for k in range(K2):
                            nc.tensor.matmul(
                                psum[:],
                                w2_sb[:, d * dw + k * 128: d * dw + (k + 1) * 128],
                                h_tiles[k][:],
                                start=(k == 0), stop=(k == K2 - 1),
                            )
                        o_sb = opool.tile([128, TOK_BLK], mybir.dt.float32,
                                          tag="o", name=f"o_sb{t}_{d}")
                        nc.scalar.activation(o_sb[:], psum[:],
                                             mybir.ActivationFunctionType.Identity,
                                             bias=b2_sb[:, d:d + 1], scale=1.0)
                        nc.scalar.dma_start(yT[d * 128:(d + 1) * 128,
                                               t * TOK_BLK:(t + 1) * TOK_BLK],
                                            o_sb[:])
                    else:
                        # final tile: two half-column groups so the last
                        # activation+DMA overlaps the second group's matmuls
                        for half in range(2):
                            c0 = half * (TOK_BLK // 2)
                            psum = ps2.tile([128, TOK_BLK], mybir.dt.float32,
                                            tag="ps2", name=f"ps2_{t}_{d}_h{half}")
                            for k in range(K2):
                                nc.tensor.matmul(
                                    psum[:, :TOK_BLK // 2],
                                    w2_sb[:, d * dw + k * 128: d * dw + (k + 1) * 128],
                                    h_tiles[k][:, c0:c0 + TOK_BLK // 2],
                                    start=(k == 0), stop=(k == K2 - 1),
                                )
                            o_sb = opool.tile([128, TOK_BLK // 2], mybir.dt.float32,
                                              tag="o", name=f"o_sb{t}_{d}_h{half}")
                            nc.scalar.activation(o_sb[:], psum[:, :TOK_BLK // 2],
                                                 mybir.ActivationFunctionType.Identity,
                                                 bias=b2_sb[:, d:d + 1], scale=1.0)
                            # split the final flight across both rings
                            q = TOK_BLK // 4
                            nc.scalar.dma_start(
                                yT[d * 128:(d + 1) * 128,
                                   t * TOK_BLK + c0: t * TOK_BLK + c0 + q],
                                o_sb[:, :q])
                            nc.sync.dma_start(
                                yT[d * 128:(d + 1) * 128,
                                   t * TOK_BLK + c0 + q: t * TOK_BLK + c0 + 2 * q],
                                o_sb[:, q:])

    nc.compile()
    return nc


def _get_compiled():
    global _compiled
    if _compiled is None:
        _compiled = _build()
    return _compiled


def _q8(a):
    return np.clip(a, -240.0, 240.0).astype(NP_F8)


def _make_in_maps(x, w1, b1, w2, b2):
    in_maps = []
    for e in range(E):
        xe = x[e * T_PER_E:(e + 1) * T_PER_E].astype(np.float32)  # [2048, 1024]
        xl = xe.reshape(N_TOK_BLK, TOK_BLK, K1, 128)     # t, c, k, p
        xl = xl.transpose(3, 0, 2, 1).reshape(128, -1)   # p, (t k c)
        # xP8 covers d_in 0..255 only: columns [t, i, c]
        x8 = xe[:, :256].reshape(N_TOK_BLK, TOK_BLK, 2, 128)
        x8 = x8.transpose(3, 0, 2, 1).reshape(128, -1)   # p, (t i c)
        w1e = (w1[e].astype(np.float32) * W1_SCALE).reshape(M1, 128, K1, 128)
        w1l = w1e.transpose(3, 0, 2, 1).reshape(128, -1)  # p, (m k mc)
        w1p = (w1[e][:NS_FP8 * 128, :256].astype(np.float32) * W8_SCALE)
        w1p = w1p.reshape(NS_FP8, 128, 2, 128)           # m, mc, i, p
        w1p = w1p.transpose(3, 0, 2, 1).reshape(128, -1)  # p, (m i mc)
        w2e = w2[e].reshape(M2, 128, K2, 128)            # d, dc, k, p
        w2l = w2e.transpose(3, 0, 2, 1).reshape(128, -1)  # p, (d k dc)
        in_maps.append({
            "xL": np.ascontiguousarray(xl).astype(NP_CDT),
            "xP8": _q8(np.ascontiguousarray(x8) * X8_SCALE),
            "w1L": np.ascontiguousarray(w1l).astype(NP_CDT),
            "w1P8": _q8(np.ascontiguousarray(w1p)),
            "w2L": np.ascontiguousarray(w2l).astype(NP_CDT),
            "b1r": np.ascontiguousarray(b1[e].reshape(M1, 128).T).astype(np.float32),
            "b2r": np.ascontiguousarray(b2[e].reshape(M2, 128).T).astype(np.float32),
        })
    return in_maps


def run(x, cnt, w1, b1, w2, b2, trace=False):
    nc = _get_compiled()
    in_maps = _make_in_maps(x, w1, b1, w2, b2)
    res = run_bass_kernel_spmd(nc, in_maps, core_ids=list(range(E)), trace=trace)
    outs = [res.results[e]["yT"].T for e in range(E)]
    y = np.concatenate(outs, axis=0).astype(np.float32)
    return y, res


def kernel(x, cnt, w1, b1, w2, b2):
    y, _ = run(x, cnt, w1, b1, w2, b2, trace=False)
    return y


# revision 12
# speedup vs baseline: 1.0509x; 1.0070x over previous
"""Expert-parallel MoE MLP (8 experts -> 8 NeuronCores) Bass kernel for TRN2.

Problem: y[t] = W2[e] @ gelu(W1[e] @ x[t] + b1[e]) + b2[e], tokens contiguous
per expert, 2048 tokens/expert, d_in=d_out=1024, d_hid=4096.

Sharding: expert-parallel. Core e gets expert e's weights and its 2048 tokens.
No cross-core communication needed (counts are equal and tokens are already
sorted by expert); host does the shard/unshard.

Per-core compute layout (everything partition-major, h kept as [hid, tok]):
  GEMM1: h[hid, tok]  = w1T[k,:].T @ xT[k, tok]   (accum over k = d_in tiles)
  GELU : h = gelu(h/512 + b1)  via ScalarE with fused per-partition bias
  GEMM2: y[dout, tok] = w2T[k,:].T @ h[k, tok]    (accum over k = d_hid tiles)
  BIAS : y += b2 via ScalarE Identity with fused bias

Measured structure: ~7us framework preamble gates everything; steady-state
fp16 MM spacing is 215.6ns (silicon floor for N=512); so the levers are the
ramp-in, HAM warmth, the serial tail, and shaving PE-cycles via fp8.

Fractional fp8 (DoubleRow): for the first NS_FP8 of 32 GEMM1 m-tiles in
token blocks 1-3 (block 0 stays fp16: DR + the concurrent ~300GB/s weight
prefetch trips the clock throttler), the d_in 0..255 contraction slice runs
as ONE fp8 DoubleRow matmul (2 MACs/cell/cycle) instead of two fp16 matmuls. Operands are quantized on the HOST
(fp8(4x), fp8(128*w1), e4m3), accumulated in fp32 PSUM; the fp16 k-slices use
w1 pre-scaled by 512 (exact power-of-2 in fp16), so the whole PSUM carries a
uniform x512 factor that the gelu evacuation removes via scale=1/512. Zero
extra on-chip ops. Offline-simulated max-rel error vs the fp32 reference:
1.41e-2 at NS_FP8=28 (gate: 2e-2); the fp8 values are host-generated so the
simulation is exact up to fp32 accumulation order.
"""
import sys

sys.path.insert(0, "/opt/trn_rl_repo")

import numpy as np
import ml_dtypes

import concourse.bass as bass  # noqa: F401
import concourse.tile as tile
from concourse import bacc, mybir
from concourse.bass_utils import run_bass_kernel_spmd

E = 8
T_PER_E = 2048
D_IN = 1024
D_HID = 4096
D_OUT = 1024

TOK_BLK = 512          # tokens per block (= PSUM bank free size in fp32)
N_TOK_BLK = T_PER_E // TOK_BLK
K1 = D_IN // 128       # k tiles for GEMM1
M1 = D_HID // 128      # output row tiles for GEMM1
K2 = D_HID // 128      # k tiles for GEMM2
M2 = D_OUT // 128      # output row tiles for GEMM2

CDT = mybir.dt.float16   # compute dtype on device (weights + activations)
NP_CDT = np.float16
F8 = mybir.dt.float8e4
NP_F8 = ml_dtypes.float8_e4m3

NS_FP8 = 32              # m-tiles whose k-pair {0,1} runs as one fp8 DR matmul
W1_SCALE = 512.0         # fp16 w1 pre-scale (psum carries x512, gelu divides)
X8_SCALE = 4.0           # fp8 x pre-scale   (4 * 128 = 512)
W8_SCALE = 128.0         # fp8 w1 pre-scale

N_WARM = 12              # N=512 warmup matmuls bridging until first data lands

_compiled = None


def _build():
    nc = bacc.Bacc("TRN2", target_bir_lowering=False, debug=False)

    # Host-permuted layouts (see _make_in_maps):
    #   xL [128, t*4096 + k*512 + c]   = x[t*512+c, k*128+p] * 1      (fp16)
    #   xP8[128, t*1024 + i*512 + c]   = x[t*512+c, i*128+p] * 4      (fp8)
    #   w1L[128, m*1024 + k*128 + mc]  = w1[m*128+mc, k*128+p] * 512  (fp16)
    #   w1P8[128, m*256 + i*128 + mc]  = w1[m*128+mc, i*128+p] * 128  (fp8)
    #   w2L[128, d*4096 + k*128 + dc]  = w2[d*128+dc, k*128+p]        (fp16)
    xL = nc.dram_tensor("xL", [128, N_TOK_BLK * K1 * TOK_BLK], CDT, kind="ExternalInput").ap()
    xP8 = nc.dram_tensor("xP8", [128, N_TOK_BLK * 2 * TOK_BLK], F8, kind="ExternalInput").ap()
    w1L = nc.dram_tensor("w1L", [128, M1 * K1 * 128], CDT, kind="ExternalInput").ap()
    w1P8 = nc.dram_tensor("w1P8", [128, NS_FP8 * 256], F8, kind="ExternalInput").ap()
    w2L = nc.dram_tensor("w2L", [128, M2 * K2 * 128], CDT, kind="ExternalInput").ap()
    b1r = nc.dram_tensor("b1r", [128, M1], mybir.dt.float32, kind="ExternalInput").ap()
    b2r = nc.dram_tensor("b2r", [128, M2], mybir.dt.float32, kind="ExternalInput").ap()
    yT = nc.dram_tensor("yT", [D_OUT, T_PER_E], mybir.dt.float32, kind="ExternalOutput").ap()

    XBLK = K1 * TOK_BLK   # 4096 fp16 cols per token block in xL
    X8BLK = 2 * TOK_BLK   # 1024 fp8 cols per token block in xP8
    mw = K1 * 128         # 1024 cols per w1 m-tile
    dw = K2 * 128         # 4096 cols per w2 d-tile

    with tile.TileContext(nc) as tc:
        with tc.tile_pool(name="wpool", bufs=1) as wpool, \
             tc.tile_pool(name="xpool", bufs=2) as xpool, \
             tc.tile_pool(name="x8pool", bufs=2) as x8pool, \
             tc.tile_pool(name="hpool", bufs=1) as hpool, \
             tc.tile_pool(name="opool", bufs=4) as opool, \
             tc.tile_pool(name="ps1", bufs=3, space="PSUM") as ps1, \
             tc.tile_pool(name="ps2", bufs=4, space="PSUM") as ps2:

            # --- PE warmup: keep the HAM clock gate warm until data lands ---
            scr = wpool.tile([128, TOK_BLK], CDT, name="scr")
            nc.vector.memset(scr[:], 0.0)
            for i in range(N_WARM):
                wps = ps1.tile([128, TOK_BLK], mybir.dt.float32, tag="ps1", name=f"warm{i}")
                nc.tensor.matmul(wps[:], scr[:, :128], scr[:], start=True, stop=True)

            # --- prologue: block 0 runs pure fp16 (the fp8 DR matmuls are
            # scoped to blocks 1-3: DR + the concurrent ~300GB/s weight
            # stream trips the clock throttler; blocks 1-3 have light DMA).
            # Critical set = x0 (1MB) + w1 m0/m1, split across BOTH HWDGE
            # rings (sync + scalar) so it drains at full HBM rate ---
            w1_sb = wpool.tile([128, M1 * mw], CDT, name="w1_sb")
            w1p8_sb = wpool.tile([128, NS_FP8 * 256], F8, name="w1p8_sb")
            x_blocks = {}
            x8_blocks = {}
            x_sb = xpool.tile([128, XBLK], CDT, tag="x", name="x_sb0")
            qx = XBLK // 4

            nc.sync.dma_start(w1_sb[:, 0:mw], w1L[:, 0:mw])                    # m0
            nc.scalar.dma_start(x_sb[:, 0:qx], xL[:, 0:qx])                    # x0 k01
            nc.sync.dma_start(w1_sb[:, mw:2 * mw], w1L[:, mw:2 * mw])          # m1
            nc.scalar.dma_start(x_sb[:, qx:2 * qx], xL[:, qx:2 * qx])          # x0 k23
            nc.sync.dma_start(x_sb[:, 2 * qx:3 * qx], xL[:, 2 * qx:3 * qx])    # x0 k45
            nc.scalar.dma_start(x_sb[:, 3 * qx:], xL[:, 3 * qx:XBLK])          # x0 k67
            x_blocks[0] = x_sb

            b1_sb = wpool.tile([128, M1], mybir.dt.float32, name="b1_sb")
            nc.sync.dma_start(b1_sb[:], b1r[:, :])
            b2_sb = wpool.tile([128, M2], mybir.dt.float32, name="b2_sb")
            nc.sync.dma_start(b2_sb[:], b2r[:, :])

            for m in range(2, M1):
                nc.sync.dma_start(w1_sb[:, m * mw:(m + 1) * mw],
                                  w1L[:, m * mw:(m + 1) * mw])

            x_sb1 = xpool.tile([128, XBLK], CDT, tag="x", name="x_sb1")
            for j in range(2):
                w = XBLK // 2
                nc.scalar.dma_start(x_sb1[:, j * w:(j + 1) * w],
                                    xL[:, XBLK + j * w: XBLK + (j + 1) * w])
            x_blocks[1] = x_sb1
            x8_sb1 = x8pool.tile([128, X8BLK], F8, tag="x8", name="x8_sb1")
            nc.scalar.dma_start(x8_sb1[:], xP8[:, X8BLK:2 * X8BLK])
            x8_blocks[1] = x8_sb1

            # --- sync ring: w2 (one d-tile per piece), then the fp8 w1
            # (first needed at G1(t=1), ~125us in) ---
            w2_sb = wpool.tile([128, M2 * dw], CDT, name="w2_sb")
            for d in range(M2):
                nc.sync.dma_start(w2_sb[:, d * dw:(d + 1) * dw],
                                  w2L[:, d * dw:(d + 1) * dw])
            nc.sync.dma_start(w1p8_sb[:], w1P8[:, :])

            for t in range(N_TOK_BLK):
                if t in x_blocks:
                    x_sb = x_blocks[t]
                else:
                    x8_sb = x8pool.tile([128, X8BLK], F8, tag="x8", name=f"x8_sb{t}")
                    nc.scalar.dma_start(x8_sb[:], xP8[:, t * X8BLK:(t + 1) * X8BLK])
                    x8_blocks[t] = x8_sb
                    x_sb = xpool.tile([128, XBLK], CDT, tag="x", name=f"x_sb{t}")
                    for j in range(2):
                        w = XBLK // 2
                        nc.scalar.dma_start(x_sb[:, j * w:(j + 1) * w],
                                            xL[:, t * XBLK + j * w: t * XBLK + (j + 1) * w])

                use_dr = t >= 1
                if use_dr:
                    x8_ap = x8_blocks[t][:, :].rearrange("p (i n) -> p i n", i=2)

                # --- GEMM1 + gelu: h[m] tiles ---
                h_tiles = []
                for m in range(M1):
                    psum = ps1.tile([128, TOK_BLK], mybir.dt.float32,
                                    tag="ps1", name=f"ps1_{t}_{m}")
                    if use_dr and m < NS_FP8:
                        w8_ap = w1p8_sb[:, m * 256:(m + 1) * 256].rearrange(
                            "p (i c) -> p i c", i=2)
                        nc.tensor.matmul(psum[:], w8_ap, x8_ap,
                                         start=True, stop=False,
                                         perf_mode=mybir.MatmulPerfMode.DoubleRow)
                        k_lo = 2
                    else:
                        k_lo = 0
                    for k in range(k_lo, K1):
                        nc.tensor.matmul(
                            psum[:],
                            w1_sb[:, m * mw + k * 128: m * mw + (k + 1) * 128],
                            x_sb[:, k * TOK_BLK:(k + 1) * TOK_BLK],
                            start=(k == 0 and k_lo == 0), stop=(k == K1 - 1),
                        )
                    h_sb = hpool.tile([128, TOK_BLK], CDT, tag=f"h{m}",
                                      name=f"h_sb{t}_{m}")
                    nc.scalar.activation(h_sb[:], psum[:],
                                         mybir.ActivationFunctionType.Gelu,
                                         bias=b1_sb[:, m:m + 1], scale=1.0 / W1_SCALE)
                    h_tiles.append(h_sb)

                # --- GEMM2 + bias: y[d] tiles ---
                for d in range(M2):
                    last = (t == N_TOK_BLK - 1 and d == M2 - 1)
                    if not last:
                        psum = ps2.tile([128, TOK_BLK], mybir.dt.float32,
                                        tag="ps2", name=f"ps2_{t}_{d}")
                        for k in range(K2):
                            nc.tensor.matmul(
                                psum[:],
                                w2_sb[:, d * dw + k * 128: d * dw + (k + 1) * 128],
                                h_tiles[k][:],
                                start=(k == 0), stop=(k == K2 - 1),
                            )
                        o_sb = opool.tile([128, TOK_BLK], mybir.dt.float32,
                                          tag="o", name=f"o_sb{t}_{d}")
                        nc.scalar.activation(o_sb[:], psum[:],
                                             mybir.ActivationFunctionType.Identity,
                                             bias=b2_sb[:, d:d + 1], scale=1.0)
                        nc.scalar.dma_start(yT[d * 128:(d + 1) * 128,
                                               t * TOK_BLK:(t + 1) * TOK_BLK],
                                            o_sb[:])
                    else:
                        # final tile: two half-column groups so the last
                        # activation+DMA overlaps the second group's matmuls
                        for half in range(2):
                            c0 = half * (TOK_BLK // 2)
                            psum = ps2.tile([128, TOK_BLK], mybir.dt.float32,
                                            tag="ps2", name=f"ps2_{t}_{d}_h{half}")
                            for k in range(K2):
                                nc.tensor.matmul(
                                    psum[:, :TOK_BLK // 2],
                                    w2_sb[:, d * dw + k * 128: d * dw + (k + 1) * 128],
                                    h_tiles[k][:, c0:c0 + TOK_BLK // 2],
                                    start=(k == 0), stop=(k == K2 - 1),
                                )
                            o_sb = opool.tile([128, TOK_BLK // 2], mybir.dt.float32,
                                              tag="o", name=f"o_sb{t}_{d}_h{half}")
                            nc.scalar.activation(o_sb[:], psum[:, :TOK_BLK // 2],
                                                 mybir.ActivationFunctionType.Identity,
                                                 bias=b2_sb[:, d:d + 1], scale=1.0)
                            # split the final flight across both rings
                            q = TOK_BLK // 4
                            nc.scalar.dma_start(
                                yT[d * 128:(d + 1) * 128,
                                   t * TOK_BLK + c0: t * TOK_BLK + c0 + q],
                                o_sb[:, :q])
                            nc.sync.dma_start(
                                yT[d * 128:(d + 1) * 128,
                                   t * TOK_BLK + c0 + q: t * TOK_BLK + c0 + 2 * q],
                                o_sb[:, q:])

    nc.compile()
    return nc


def _get_compiled():
    global _compiled
    if _compiled is None:
        _compiled = _build()
    return _compiled


def _q8(a):
    return np.clip(a, -240.0, 240.0).astype(NP_F8)


def _make_in_maps(x, w1, b1, w2, b2):
    in_maps = []
    for e in range(E):
        xe = x[e * T_PER_E:(e + 1) * T_PER_E].astype(np.float32)  # [2048, 1024]
        xl = xe.reshape(N_TOK_BLK, TOK_BLK, K1, 128)     # t, c, k, p
        xl = xl.transpose(3, 0, 2, 1).reshape(128, -1)   # p, (t k c)
        # xP8 covers d_in 0..255 only: columns [t, i, c]
        x8 = xe[:, :256].reshape(N_TOK_BLK, TOK_BLK, 2, 128)
        x8 = x8.transpose(3, 0, 2, 1).reshape(128, -1)   # p, (t i c)
        w1e = (w1[e].astype(np.float32) * W1_SCALE).reshape(M1, 128, K1, 128)
        w1l = w1e.transpose(3, 0, 2, 1).reshape(128, -1)  # p, (m k mc)
        w1p = (w1[e][:NS_FP8 * 128, :256].astype(np.float32) * W8_SCALE)
        w1p = w1p.reshape(NS_FP8, 128, 2, 128)           # m, mc, i, p
        w1p = w1p.transpose(3, 0, 2, 1).reshape(128, -1)  # p, (m i mc)
        w2e = w2[e].reshape(M2, 128, K2, 128)            # d, dc, k, p
        w2l = w2e.transpose(3, 0, 2, 1).reshape(128, -1)  # p, (d k dc)
        in_maps.append({
            "xL": np.ascontiguousarray(xl).astype(NP_CDT),
            "xP8": _q8(np.ascontiguousarray(x8) * X8_SCALE),
            "w1L": np.ascontiguousarray(w1l).astype(NP_CDT),
            "w1P8": _q8(np.ascontiguousarray(w1p)),
            "w2L": np.ascontiguousarray(w2l).astype(NP_CDT),
            "b1r": np.ascontiguousarray(b1[e].reshape(M1, 128).T).astype(np.float32),
            "b2r": np.ascontiguousarray(b2[e].reshape(M2, 128).T).astype(np.float32),
        })
    return in_maps


def run(x, cnt, w1, b1, w2, b2, trace=False):
    nc = _get_compiled()
    in_maps = _make_in_maps(x, w1, b1, w2, b2)
    res = run_bass_kernel_spmd(nc, in_maps, core_ids=list(range(E)), trace=trace)
    outs = [res.results[e]["yT"].T for e in range(E)]
    y = np.concatenate(outs, axis=0).astype(np.float32)
    return y, res


def kernel(x, cnt, w1, b1, w2, b2):
    y, _ = run(x, cnt, w1, b1, w2, b2, trace=False)
    return y
